# revision 12
# baseline (speedup 1.0000x reference)
"""Trainium2 Bass kernel for LocalFeatureSamplerV10 (retrieval_knn), v3.

Full-input contract: kernel(**inputs) takes the complete unsharded numpy
inputs and returns the full [32, 512] output. Internally shards the batch
dim over 8 NeuronCores (4 batches/core), replicating the MLP weights.

v3 changes vs v2 (98.6us measured):
  * PE HAM warm-up: dummy matmul streams anchored on time-spread producers
    (distance reduces, topk rounds, find groups, gather results) keep the
    PE clock gate open through the KNN phase so the MLP runs at full rate
    (v2 MLP matmuls measured 756ns vs ~250ns warm).
  * Winner broadcast via 8 selector matmuls (erep) instead of a DMA hop.
  * Stage C->D flatten via PE transposes instead of a DMA hop.
  * Index tables built with one masked matmul (Lrep.T @ (gcl*mask8))
    replacing the DMA + transpose + replicate chain per chunk.
  * Distance phase split: 5 pairs on vector, 3 on gpsimd.
  * Points staged in 2 DMAs so batches 0/1 start ~1us earlier.

Per-core algorithm (4 batches x 2 queries = 8 "pairs", pair = t*4 + b):
  1. s = -||p - q||^2 laid out [128 part, 128] per pair (point n = p*128+j).
  2. Top-32 per pair: per-partition top-8 (max8) -> PE-transpose candidates
     -> per-row top-32 (max + match_replace rounds) -> PE-transpose flatten
     -> global top-32 with the 8 pairs stacked on partitions (bit-exact).
  3. Indices via max_index against the original s rows + p*128, cross-
     partition min via PE transpose + reduce_min, clamped, + batch offset;
     FIND outputs land in permuted columns so the result is already in
     dma_gather's wrapped [16, n/16] table order.
  4. Two dma_gather(transpose=True) of 128 rows each from the bf16 feature
     stacks; vector reduce_max over K -> X [128ch, 8chhi, b, t] bf16.
  5. MLPs as bf16 PE matmuls with batch on partitions; biases folded in as
     rank-1 ones-matmuls; PE transposes between layers; fp32 output.
"""

import numpy as np
import ml_dtypes

import concourse.bass as bass
from concourse import bacc
import concourse.mybir as mybir
import concourse.tile as tile

B, N, C, K, OUT = 32, 16384, 1024, 32, 512
H = 512
NCORES = 8
BPC = B // NCORES          # batches per core
P = 128
NP = N // P                # 128 points per partition
NPAIR = 2 * BPC            # 8 (pair = t*BPC + b; 0-3 joint, 4-7 drag)
F32 = mybir.dt.float32
BF16 = mybir.dt.bfloat16
U32 = mybir.dt.uint32
I16 = mybir.dt.int16
NEG = -3.0e38

AX = mybir.AxisListType
OP = mybir.AluOpType
ACTF = mybir.ActivationFunctionType

BF = ml_dtypes.bfloat16


def build_nc():
    nc = bacc.Bacc(trn_type="TRN2")

    pts = nc.dram_tensor("pts", [P, BPC * NP * 3], F32, kind="ExternalInput")
    qb = nc.dram_tensor("qb", [P, NPAIR * 3], F32, kind="ExternalInput")
    identf = nc.dram_tensor("identf", [P, P], F32, kind="ExternalInput")
    identb = nc.dram_tensor("identb", [P, P], BF16, kind="ExternalInput")
    onesb = nc.dram_tensor("onesb", [1, P], BF16, kind="ExternalInput")
    pbase = nc.dram_tensor("pbase", [P, 1], F32, kind="ExternalInput")
    boffs = nc.dram_tensor("boffs", [P, 1], F32, kind="ExternalInput")
    mask8 = nc.dram_tensor("mask8", [P, 8], F32, kind="ExternalInput")
    lrep = nc.dram_tensor("lrep", [P, P], F32, kind="ExternalInput")
    erep = nc.dram_tensor("erep", [NPAIR, NPAIR * P], F32, kind="ExternalInput")
    maskr = nc.dram_tensor("maskr", [NPAIR * 8, 8], F32, kind="ExternalInput")
    selq = nc.dram_tensor("selq", [NPAIR * 8, NPAIR], F32, kind="ExternalInput")
    feats = [nc.dram_tensor(f"feats{h}", [2 * N, C], BF16, kind="ExternalInput")
             for h in range(2)]
    wd = {}
    for t in range(3):
        wd[f"w1_{t}"] = nc.dram_tensor(f"w1_{t}", [P, 8 * H], BF16,
                                       kind="ExternalInput")
        wd[f"w2_{t}"] = nc.dram_tensor(f"w2_{t}", [P, 4 * H], BF16,
                                       kind="ExternalInput")
        wd[f"b1_{t}"] = nc.dram_tensor(f"b1_{t}", [1, H], BF16,
                                       kind="ExternalInput")
        wd[f"b2_{t}"] = nc.dram_tensor(f"b2_{t}", [1, H], BF16,
                                       kind="ExternalInput")
    out = nc.dram_tensor("out", [BPC, OUT], F32, kind="ExternalOutput")

    with tile.TileContext(nc) as tc:
        _body(tc, nc, pts, qb, identf, identb, onesb, pbase, boffs,
              mask8, lrep, erep, maskr, selq, feats, wd, out)
    nc.compile()
    return nc


def _body(tc, nc, pts, qb, identf, identb, onesb, pbase, boffs,
          mask8, lrep, erep, maskr, selq, feats, wd, out):
    from contextlib import ExitStack
    with ExitStack() as ctx:
        cpool = ctx.enter_context(tc.tile_pool(name="const", bufs=1))
        wpool = ctx.enter_context(tc.tile_pool(name="weights", bufs=1))
        state = ctx.enter_context(tc.tile_pool(name="state", bufs=1))
        work = ctx.enter_context(tc.tile_pool(name="work", bufs=2))
        psA = ctx.enter_context(tc.tile_pool(name="psA", bufs=1, space="PSUM"))
        psB = ctx.enter_context(tc.tile_pool(name="psB", bufs=3, space="PSUM"))
        psT = ctx.enter_context(tc.tile_pool(name="psT", bufs=1, space="PSUM"))
        psC = ctx.enter_context(tc.tile_pool(name="psC", bufs=1, space="PSUM"))
        psD = ctx.enter_context(tc.tile_pool(name="psD", bufs=1, space="PSUM"))

        # ---- critical-path inputs first, on the sync queue ---------------
        qb_s = state.tile([P, NPAIR, 3], F32, tag="qb_s")
        nc.sync.dma_start(out=qb_s[:, :, :],
                          in_=qb[:, :].rearrange("p (i c) -> p i c", c=3))
        ptile = state.tile([P, BPC, NP * 3], F32, tag="ptile")
        for half in range(2):
            nc.sync.dma_start(
                out=ptile[:, 2 * half:2 * half + 2, :],
                in_=pts[:, :].rearrange("p (b x) -> p b x", b=BPC)
                [:, 2 * half:2 * half + 2, :])

        # ---- constants on the scalar queue -------------------------------
        ident = cpool.tile([P, P], F32, tag="ident")
        nc.gpsimd.dma_start(out=ident[:, :], in_=identf[:, :])
        identb_s = cpool.tile([P, P], BF16, tag="identb_s")
        nc.gpsimd.dma_start(out=identb_s[:, :], in_=identb[:, :])
        onesb_s = cpool.tile([1, P], BF16, tag="onesb_s")
        nc.gpsimd.dma_start(out=onesb_s[:, :], in_=onesb[:, :])
        pbase_s = cpool.tile([P, 1], F32, tag="pbase_s")
        nc.gpsimd.dma_start(out=pbase_s[:, :], in_=pbase[:, :])
        boffs_s = cpool.tile([P, 1], F32, tag="boffs_s")
        nc.gpsimd.dma_start(out=boffs_s[:, :], in_=boffs[:, :])
        mask8_s = cpool.tile([P, 8], F32, tag="mask8_s")
        nc.gpsimd.dma_start(out=mask8_s[:, :], in_=mask8[:, :])
        lrep_s = cpool.tile([P, P], F32, tag="lrep_s")
        nc.gpsimd.dma_start(out=lrep_s[:, :], in_=lrep[:, :])
        erep_s = cpool.tile([NPAIR, NPAIR * P], F32, tag="erep_s")
        nc.gpsimd.dma_start(out=erep_s[:, :], in_=erep[:, :])
        maskr_s = cpool.tile([NPAIR * 8, 8], F32, tag="maskr_s")
        nc.gpsimd.dma_start(out=maskr_s[:, :], in_=maskr[:, :])
        selq_s = cpool.tile([NPAIR * 8, NPAIR], F32, tag="selq_s")
        nc.gpsimd.dma_start(out=selq_s[:, :], in_=selq[:, :])

        # ---- bulk weights on the sync queue (behind pts) -----------------
        w1s, w2s, b1s, b2s = {}, {}, {}, {}
        for t in range(3):
            w1 = wpool.tile([P, 8, H], BF16, tag=f"w1_{t}")
            nc.sync.dma_start(out=w1[:, :, :],
                              in_=wd[f"w1_{t}"][:, :].rearrange(
                                  "p (ch o) -> p ch o", ch=8))
            w2 = wpool.tile([P, 4, H], BF16, tag=f"w2_{t}")
            nc.sync.dma_start(out=w2[:, :, :],
                              in_=wd[f"w2_{t}"][:, :].rearrange(
                                  "p (ch o) -> p ch o", ch=4))
            b1 = wpool.tile([1, H], BF16, tag=f"b1_{t}")
            nc.sync.dma_start(out=b1[:, :], in_=wd[f"b1_{t}"][:, :])
            b2 = wpool.tile([1, H], BF16, tag=f"b2_{t}")
            nc.sync.dma_start(out=b2[:, :], in_=wd[f"b2_{t}"][:, :])
            w1s[t], w2s[t], b1s[t], b2s[t] = w1, w2, b1, b2

        # ---- PE warm-up scratch -----------------------------------------
        dps = psD.tile([NPAIR, H], F32, tag="dummy", name="dps")

        def warm_f32(anchor_ap):
            """Dummy fp32 matmul keyed on anchor_ap to keep PE un-gated."""
            kk = anchor_ap.shape[0]
            nc.tensor.matmul(out=dps[:anchor_ap.shape[1], :P], lhsT=anchor_ap,
                             rhs=ident[:kk, :], start=True, stop=True)

        def warm_bf16(anchor_ap):
            nc.tensor.matmul(out=dps[:anchor_ap.shape[1], :], lhsT=anchor_ap,
                             rhs=w1s[0][:, 0, :], start=True, stop=True)

        # ---- stage A: s = -d2, stage B: per-partition top-8 --------------
        # vector handles pairs [0,1,4,2,6]; gpsimd handles [5,3,7]
        s_all = state.tile([P, NPAIR, NP], F32, tag="s_all")
        v8f = state.tile([P, NPAIR * 8], F32, tag="v8f")

        def dist_sub(eng, i):
            b = i % BPC
            pv = ptile[:, b, :].rearrange("p (j c) -> p j c", c=3)
            diff = work.tile([P, NP * 3], F32, tag="diff", name=f"diff{i}")
            eng.tensor_sub(
                out=diff[:, :].rearrange("p (j c) -> p j c", c=3), in0=pv,
                in1=qb_s[:, i:i + 1, :].to_broadcast([P, NP, 3]))
            return diff

        def dist_sqred(eng, i, diff):
            sq = work.tile([P, NP * 3], F32, tag="sq", name=f"sq{i}")
            nc.scalar.square(out=sq[:, :], in_=diff[:, :])
            eng.tensor_reduce(out=s_all[:, i, :],
                              in_=sq[:, :].rearrange("p (j c) -> p j c", c=3),
                              axis=AX.X, op=OP.add, negate=True)

        for i in [0, 1, 4, 5, 2, 3, 6, 7]:
            d = dist_sub(nc.vector, i)
            dist_sqred(nc.vector, i, d)
            nc.vector.max(out=v8f[:, i * 8:(i + 1) * 8], in_=s_all[:, i, :])

        # ---- transpose candidates: [128, 64] -> [64, 128] ----------------
        tvp = psA.tile([NPAIR * 8, P], F32, tag="t64", name="tvp")
        nc.tensor.transpose(out=tvp[:, :], in_=v8f[:, :], identity=ident[:, :])
        tv = state.tile([NPAIR * 8, P], F32, tag="tv")
        nc.vector.tensor_copy(tv[:, :], tvp[:, :])

        # ---- stage C: per-row top-32 of candidates -----------------------
        cv = state.tile([NPAIR * 8, 32], F32, tag="cv")
        for r in range(4):
            sl = cv[:, r * 8:(r + 1) * 8]
            nc.vector.max(out=sl, in_=tv[:, :])
            if r < 3:
                nc.vector.match_replace(out=tv[:, :], in_to_replace=sl,
                                        in_values=tv[:, :], imm_value=NEG)

        # ---- flatten [64,32] -> [8,256] via one masked matmul ------------
        # cvmask[k, r*32+c] = cv[k, c]*(k%8==r); cand = selq.T @ cvmask so
        # cand[q, r*32+c] = cv[q*8+r, c].
        cvmask = state.tile([NPAIR * 8, 8, 32], F32, tag="cvmask")
        nc.vector.tensor_tensor(
            out=cvmask[:, :, :],
            in0=cv[:, :].rearrange("k (a c) -> k a c", a=1).to_broadcast(
                [NPAIR * 8, 8, 32]),
            in1=maskr_s[:, :].rearrange("k (r u) -> k r u", u=1).to_broadcast(
                [NPAIR * 8, 8, 32]),
            op=OP.mult)
        candp = psC.tile([NPAIR, NPAIR * 32], F32, tag="ctr", name="candp")
        nc.tensor.matmul(out=candp[:, :], lhsT=selq_s[:, :],
                         rhs=cvmask[:, :, :].rearrange("k a c -> k (a c)"),
                         start=True, stop=True)
        cand = state.tile([NPAIR, 8 * 32], F32, tag="cand")
        nc.vector.tensor_copy(cand[:, :], candp[:, :])

        # ---- stage D: global top-32 --------------------------------------
        wv = state.tile([NPAIR, 32], F32, tag="wv")
        for r in range(4):
            sl = wv[:, r * 8:(r + 1) * 8]
            nc.vector.max(out=sl, in_=cand[:, :])
            if r < 3:
                nc.vector.match_replace(out=cand[:, :], in_to_replace=sl,
                                        in_values=cand[:, :], imm_value=NEG)

        # ---- broadcast winners via per-pair selector matmuls, pipelined
        # with the finds (pair i's finds start as soon as its own 128x32
        # broadcast lands; emission order matches the find loop) ----------
        wBs = {}
        for q in [0, 1, 4, 5, 2, 3, 6, 7]:
            wbp = psA.tile([P, 32], F32, tag="bc", name=f"wbp{q}")
            nc.tensor.matmul(out=wbp[:, :],
                             lhsT=erep_s[:, q * P:(q + 1) * P],
                             rhs=wv[:, :], start=True, stop=True)
            wB = state.tile([P, 32], F32, tag=f"wB{q}", name=f"wB{q}")
            nc.vector.tensor_copy(wB[:, :], wbp[:, :])
            wBs[q] = wB

        # ---- per 2-batch chunk: index recovery + gather + maxpool --------
        # ju column for (pair, g): h*128 + b2*64 + t*32 + (g//2)*16 + (g%2)*8
        # so that post-transpose partition q = b2*64 + t*32 + w, which is
        # dma_gather's unwrapped slot order (slot i reads table[i%16, i//16])
        # once the masked Lrep matmul rewraps gcl into table[k,j]=gcl[j*16+k]
        # replicated to every 16-partition block.
        jus = [state.tile([P, P], U32, tag=f"ju{h}", name=f"ju{h}")
               for h in range(2)]
        jfs = [state.tile([P, P], F32, tag=f"jf{h}", name=f"jf{h}")
               for h in range(2)]
        gfins = [state.tile([P, 1], F32, tag=f"gfin{h}", name=f"gfin{h}")
                 for h in range(2)]
        gcls = [state.tile([P, 1], F32, tag=f"gcl{h}", name=f"gcl{h}")
                for h in range(2)]
        Xall = state.tile([P, 8, BPC, 2], BF16, tag="Xall")
        gmasks = []
        xgs = []
        for hh in range(2):
            for t in range(2):
                for b2 in range(2):
                    i = t * BPC + 2 * hh + b2
                    for g in range(4):
                        col = b2 * 64 + t * 32 + (g // 2) * 16 + (g % 2) * 8
                        nc.vector.max_index(out=jus[hh][:, col:col + 8],
                                            in_max=wBs[i][:, g * 8:(g + 1) * 8],
                                            in_values=s_all[:, i, :])
            jfh = jfs[hh][:, :]
            nc.vector.tensor_copy(jfh, jus[hh][:, :])
            nc.vector.scalar_tensor_tensor(
                out=jfh, in0=jfh, scalar=1.0,
                in1=pbase_s[:, :].to_broadcast([P, P]),
                op0=OP.mult, op1=OP.add)
            tp = psA.tile([P, P], F32, tag="t64", name=f"tp{hh}")
            nc.tensor.transpose(out=tp[:, :], in_=jfh, identity=ident[:, :])
            nc.vector.tensor_reduce(out=gfins[hh][:, :], in_=tp[:, :],
                                    axis=AX.X, op=OP.min)
            # clamp NOT_FOUND (huge) to N-1 and add per-slot batch offset
            nc.vector.scalar_tensor_tensor(
                out=gcls[hh][:, :], in0=gfins[hh][:, :],
                scalar=float(N - 1), in1=boffs_s[:, :],
                op0=OP.min, op1=OP.add)
            # wrapped+replicated idx table in one masked matmul
            gmask = state.tile([P, 8], F32, tag=f"gmask{hh}", name=f"gmask{hh}")
            gmasks.append(gmask)
            nc.vector.tensor_tensor(
                out=gmask[:, :], in0=gcls[hh][:, :].to_broadcast([P, 8]),
                in1=mask8_s[:, :], op=OP.mult)
            Tp = psA.tile([P, 8], F32, tag="bc", name=f"Tp{hh}")
            nc.tensor.matmul(out=Tp[:, :], lhsT=lrep_s[:, :], rhs=gmask[:, :],
                             start=True, stop=True)
            idx16 = state.tile([P, 8], I16, tag=f"idx16_{hh}", name=f"idx16_{hh}")
            nc.vector.tensor_copy(idx16[:, :], Tp[:, :])
            xg = state.tile([P, 8, P], BF16, tag=f"xg{hh}", name=f"xg{hh}")
            nc.gpsimd.dma_gather(
                xg[:, :, :], feats[hh][:, :], idx16[:, :],
                num_idxs=P, num_idxs_reg=P, elem_size=C, transpose=True)
            xgs.append(xg)

        for hh in range(2):
            nc.vector.tensor_reduce(
                out=Xall[:, :, 2 * hh:2 * hh + 2, :],
                in_=xgs[hh][:, :, :].rearrange("p c8 (b2 t w) -> p c8 b2 t w",
                                               t=2, w=32),
                axis=AX.X, op=OP.max)

        # solid PE warm-up block across the gather wait: first dummy is
        # gated on chunk 1's index table (so the block can't be hoisted
        # early and re-cool), the rest self-chain via WAR on dps and run
        # back-to-back, lifting the HAM clock gate before the MLP.
        warm_f32(gmasks[1][:, :])
        for _ in range(16):
            warm_bf16(identb_s[:, :8])

        # ---- MLPs (bf16) -------------------------------------------------
        def mlp2(t, xin_sl):
            """xin_sl(ch) -> lhsT [128, BPC] bf16; returns psum [BPC, H]."""
            ps1 = psB.tile([BPC, H], F32, tag="mm", name=f"ps1_{t}")
            for ch in range(8):
                nc.tensor.matmul(out=ps1[:, :], lhsT=xin_sl(ch),
                                 rhs=w1s[t][:, ch, :], start=(ch == 0), stop=False)
            nc.tensor.matmul(out=ps1[:, :], lhsT=onesb_s[:1, :BPC],
                             rhs=b1s[t][:1, :], start=False, stop=True)
            h = state.tile([BPC, H], BF16, tag=f"h_{t}")
            nc.scalar.activation(out=h[:, :], in_=ps1[:, :], func=ACTF.Relu)
            hTp = psT.tile([P, 4 * BPC], BF16, tag="tr", name=f"hTp_{t}")
            for ic in range(4):
                nc.tensor.transpose(out=hTp[:, ic * BPC:(ic + 1) * BPC],
                                    in_=h[:, ic * P:(ic + 1) * P],
                                    identity=identb_s[:BPC, :BPC])
            hT = state.tile([P, 4, BPC], BF16, tag=f"hT_{t}")
            nc.vector.tensor_copy(hT[:, :, :],
                                  hTp[:, :].rearrange("p (ic b) -> p ic b", b=BPC))
            ps2 = psB.tile([BPC, H], F32, tag="mm", name=f"ps2_{t}")
            for ic in range(4):
                nc.tensor.matmul(out=ps2[:, :], lhsT=hT[:, ic, :],
                                 rhs=w2s[t][:, ic, :], start=(ic == 0), stop=False)
            nc.tensor.matmul(out=ps2[:, :], lhsT=onesb_s[:1, :BPC],
                             rhs=b2s[t][:1, :], start=False, stop=True)
            return ps2

        # f-MLP layer 1 accumulates per concat-half so the joint half runs
        # while the drag MLP is still in flight
        cTp = psC.tile([P, 8 * BPC], BF16, tag="ctr", name="cTp")
        psf1 = psB.tile([BPC, H], F32, tag="mm", name="psf1")
        for t in range(2):
            ps2 = mlp2(t, lambda ch: Xall[:, ch, :, t])
            o = state.tile([BPC, H], BF16, tag=f"o_{t}")
            nc.scalar.activation(out=o[:, :], in_=ps2[:, :], func=ACTF.Copy)
            for ic in range(4):
                nc.tensor.transpose(
                    out=cTp[:, (t * 4 + ic) * BPC:(t * 4 + ic + 1) * BPC],
                    in_=o[:, ic * P:(ic + 1) * P],
                    identity=identb_s[:BPC, :BPC])
            cTh = state.tile([P, 4, BPC], BF16, tag=f"cT{t}", name=f"cT{t}")
            nc.vector.tensor_copy(
                cTh[:, :, :],
                cTp[:, t * 4 * BPC:(t + 1) * 4 * BPC].rearrange(
                    "p (ic b) -> p ic b", b=BPC))
            for ic in range(4):
                nc.tensor.matmul(out=psf1[:, :], lhsT=cTh[:, ic, :],
                                 rhs=w1s[2][:, t * 4 + ic, :],
                                 start=(t == 0 and ic == 0), stop=False)
        nc.tensor.matmul(out=psf1[:, :], lhsT=onesb_s[:1, :BPC],
                         rhs=b1s[2][:1, :], start=False, stop=True)
        hf = state.tile([BPC, H], BF16, tag="h_f")
        nc.scalar.activation(out=hf[:, :], in_=psf1[:, :], func=ACTF.Relu)
        hTpf = psT.tile([P, 4 * BPC], BF16, tag="tr", name="hTp_f")
        for ic in range(4):
            nc.tensor.transpose(out=hTpf[:, ic * BPC:(ic + 1) * BPC],
                                in_=hf[:, ic * P:(ic + 1) * P],
                                identity=identb_s[:BPC, :BPC])
        hTf = state.tile([P, 4, BPC], BF16, tag="hT_f")
        nc.vector.tensor_copy(hTf[:, :, :],
                              hTpf[:, :].rearrange("p (ic b) -> p ic b", b=BPC))
        psf2 = psB.tile([BPC, H], F32, tag="mm", name="psf2")
        for ic in range(4):
            nc.tensor.matmul(out=psf2[:, :], lhsT=hTf[:, ic, :],
                             rhs=w2s[2][:, ic, :], start=(ic == 0), stop=False)
        nc.tensor.matmul(out=psf2[:, :], lhsT=onesb_s[:1, :BPC],
                         rhs=b2s[2][:1, :], start=False, stop=True)
        res = state.tile([BPC, OUT], F32, tag="res")
        nc.vector.tensor_copy(res[:, :], psf2[:, :])
        nc.sync.dma_start(out=out[:, :], in_=res[:, :])


_NC_CACHE = None


def _get_nc():
    global _NC_CACHE
    if _NC_CACHE is None:
        _NC_CACHE = build_nc()
    return _NC_CACHE


def _consts():
    identf = np.eye(P, dtype=np.float32)
    identb = np.eye(P).astype(BF)
    onesb = np.ones((1, P)).astype(BF)
    pbase = (np.arange(P, dtype=np.float32) * NP).reshape(P, 1)
    boffs = ((np.arange(P) // 64) * N).astype(np.float32).reshape(P, 1)
    mask8 = (np.arange(P)[:, None] // 16 == np.arange(8)[None, :]).astype(
        np.float32)
    lrep = (np.arange(P)[:, None] % 16 == np.arange(P)[None, :] % 16).astype(
        np.float32)
    erep = np.zeros((NPAIR, NPAIR * P), dtype=np.float32)
    for q in range(NPAIR):
        erep[q, q * P:(q + 1) * P] = 1.0
    maskr = (np.arange(NPAIR * 8)[:, None] % 8 == np.arange(8)[None, :]
             ).astype(np.float32)
    selq = (np.arange(NPAIR * 8)[:, None] // 8 == np.arange(NPAIR)[None, :]
            ).astype(np.float32)
    return {"identf": identf, "identb": identb, "onesb": onesb,
            "pbase": pbase, "boffs": boffs, "mask8": mask8,
            "lrep": lrep, "erep": erep, "maskr": maskr, "selq": selq}


def build_in_maps(points_xyz, point_features, joint_origin, drag_point,
                  jw1, jb1, jw2, jb2, dw1, db1, dw2, db2, fw1, fb1, fw2, fb2):
    from concurrent.futures import ThreadPoolExecutor

    wmap = {}
    for t, (w1, b1, w2, b2) in enumerate([(jw1, jb1, jw2, jb2),
                                          (dw1, db1, dw2, db2),
                                          (fw1, fb1, fw2, fb2)]):
        w1 = np.asarray(w1, dtype=np.float32)
        w2 = np.asarray(w2, dtype=np.float32)
        nch = w1.shape[0] // P
        wmap[f"w1_{t}"] = np.ascontiguousarray(
            w1.reshape(nch, P, H).transpose(1, 0, 2).reshape(P, nch * H)
        ).astype(BF)
        wmap[f"w2_{t}"] = np.ascontiguousarray(
            w2.reshape(4, P, H).transpose(1, 0, 2).reshape(P, 4 * H)
        ).astype(BF)
        wmap[f"b1_{t}"] = np.asarray(b1, dtype=np.float32).reshape(1, H).astype(BF)
        wmap[f"b2_{t}"] = np.asarray(b2, dtype=np.float32).reshape(1, H).astype(BF)
    wmap.update(_consts())

    pxyz = np.asarray(points_xyz, dtype=np.float32)
    pf = np.asarray(point_features)
    qj = np.asarray(joint_origin, dtype=np.float32)
    qd = np.asarray(drag_point, dtype=np.float32)

    def feats_half(args):
        c, hhalf = args
        buf = np.empty((2 * N, C), dtype=BF)
        for b2 in range(2):
            gb = c * BPC + 2 * hhalf + b2
            buf[b2 * N:(b2 + 1) * N] = pf[gb].T.astype(BF)
        return buf

    with ThreadPoolExecutor(max_workers=16) as ex:
        fhalves = list(ex.map(feats_half,
                              [(c, hh) for c in range(NCORES) for hh in range(2)]))

    in_maps = []
    for c in range(NCORES):
        sl = slice(c * BPC, (c + 1) * BPC)
        ptsc = np.ascontiguousarray(
            pxyz[sl].reshape(BPC, P, NP, 3).transpose(1, 0, 2, 3)
        ).reshape(P, BPC * NP * 3)
        qcat = np.concatenate([qj[sl], qd[sl]], axis=0).reshape(-1)
        qbc = np.ascontiguousarray(
            np.broadcast_to(qcat[None, :], (P, NPAIR * 3)))
        m = {"pts": ptsc, "qb": qbc,
             "feats0": fhalves[c * 2], "feats1": fhalves[c * 2 + 1]}
        m.update(wmap)
        in_maps.append(m)
    return in_maps


def kernel(**inputs):
    from concourse import bass_utils

    nc = _get_nc()
    in_maps = build_in_maps(**inputs)
    res = bass_utils.run_bass_kernel_spmd(nc, in_maps, core_ids=list(range(NCORES)))
    return np.concatenate([r["out"] for r in res.results], axis=0)


# revision 13
# speedup vs baseline: 1.0130x; 1.0130x over previous
"""Trainium2 Bass kernel for LocalFeatureSamplerV10 (retrieval_knn), v3.

Full-input contract: kernel(**inputs) takes the complete unsharded numpy
inputs and returns the full [32, 512] output. Internally shards the batch
dim over 8 NeuronCores (4 batches/core), replicating the MLP weights.

v3 changes vs v2 (98.6us measured):
  * PE HAM warm-up: dummy matmul streams anchored on time-spread producers
    (distance reduces, topk rounds, find groups, gather results) keep the
    PE clock gate open through the KNN phase so the MLP runs at full rate
    (v2 MLP matmuls measured 756ns vs ~250ns warm).
  * Winner broadcast via 8 selector matmuls (erep) instead of a DMA hop.
  * Stage C->D flatten via PE transposes instead of a DMA hop.
  * Index tables built with one masked matmul (Lrep.T @ (gcl*mask8))
    replacing the DMA + transpose + replicate chain per chunk.
  * Distance phase split: 5 pairs on vector, 3 on gpsimd.
  * Points staged in 2 DMAs so batches 0/1 start ~1us earlier.

Per-core algorithm (4 batches x 2 queries = 8 "pairs", pair = t*4 + b):
  1. s = -||p - q||^2 laid out [128 part, 128] per pair (point n = p*128+j).
  2. Top-32 per pair: per-partition top-8 (max8) -> PE-transpose candidates
     -> per-row top-32 (max + match_replace rounds) -> PE-transpose flatten
     -> global top-32 with the 8 pairs stacked on partitions (bit-exact).
  3. Indices via max_index against the original s rows + p*128, cross-
     partition min via PE transpose + reduce_min, clamped, + batch offset;
     FIND outputs land in permuted columns so the result is already in
     dma_gather's wrapped [16, n/16] table order.
  4. Two dma_gather(transpose=True) of 128 rows each from the bf16 feature
     stacks; vector reduce_max over K -> X [128ch, 8chhi, b, t] bf16.
  5. MLPs as bf16 PE matmuls with batch on partitions; biases folded in as
     rank-1 ones-matmuls; PE transposes between layers; fp32 output.
"""

import numpy as np
import ml_dtypes

import concourse.bass as bass
from concourse import bacc
import concourse.mybir as mybir
import concourse.tile as tile

B, N, C, K, OUT = 32, 16384, 1024, 32, 512
H = 512
NCORES = 8
BPC = B // NCORES          # batches per core
P = 128
NP = N // P                # 128 points per partition
NPAIR = 2 * BPC            # 8 (pair = t*BPC + b; 0-3 joint, 4-7 drag)
F32 = mybir.dt.float32
BF16 = mybir.dt.bfloat16
U32 = mybir.dt.uint32
I16 = mybir.dt.int16
NEG = -3.0e38

AX = mybir.AxisListType
OP = mybir.AluOpType
ACTF = mybir.ActivationFunctionType

BF = ml_dtypes.bfloat16


def build_nc():
    nc = bacc.Bacc(trn_type="TRN2")

    pts = nc.dram_tensor("pts", [P, BPC * NP * 3], F32, kind="ExternalInput")
    qb = nc.dram_tensor("qb", [P, NPAIR * 3], F32, kind="ExternalInput")
    identf = nc.dram_tensor("identf", [P, P], F32, kind="ExternalInput")
    identb = nc.dram_tensor("identb", [P, P], BF16, kind="ExternalInput")
    onesb = nc.dram_tensor("onesb", [1, P], BF16, kind="ExternalInput")
    pbase = nc.dram_tensor("pbase", [P, 1], F32, kind="ExternalInput")
    boffs = nc.dram_tensor("boffs", [P, 1], F32, kind="ExternalInput")
    mask8 = nc.dram_tensor("mask8", [P, 8], F32, kind="ExternalInput")
    lrep = nc.dram_tensor("lrep", [P, P], F32, kind="ExternalInput")
    erep = nc.dram_tensor("erep", [NPAIR, NPAIR * P], F32, kind="ExternalInput")
    maskr = nc.dram_tensor("maskr", [NPAIR * 8, 8], F32, kind="ExternalInput")
    selq = nc.dram_tensor("selq", [NPAIR * 8, NPAIR], F32, kind="ExternalInput")
    feats = [nc.dram_tensor(f"feats{h}", [2 * N, C], BF16, kind="ExternalInput")
             for h in range(2)]
    wd = {}
    for t in range(3):
        wd[f"w1_{t}"] = nc.dram_tensor(f"w1_{t}", [P, 8 * H], BF16,
                                       kind="ExternalInput")
        wd[f"w2_{t}"] = nc.dram_tensor(f"w2_{t}", [P, 4 * H], BF16,
                                       kind="ExternalInput")
        wd[f"b1_{t}"] = nc.dram_tensor(f"b1_{t}", [1, H], BF16,
                                       kind="ExternalInput")
        wd[f"b2_{t}"] = nc.dram_tensor(f"b2_{t}", [1, H], BF16,
                                       kind="ExternalInput")
    out = nc.dram_tensor("out", [BPC, OUT], F32, kind="ExternalOutput")

    with tile.TileContext(nc) as tc:
        _body(tc, nc, pts, qb, identf, identb, onesb, pbase, boffs,
              mask8, lrep, erep, maskr, selq, feats, wd, out)
    nc.compile()
    return nc


def _body(tc, nc, pts, qb, identf, identb, onesb, pbase, boffs,
          mask8, lrep, erep, maskr, selq, feats, wd, out):
    from contextlib import ExitStack
    with ExitStack() as ctx:
        cpool = ctx.enter_context(tc.tile_pool(name="const", bufs=1))
        wpool = ctx.enter_context(tc.tile_pool(name="weights", bufs=1))
        state = ctx.enter_context(tc.tile_pool(name="state", bufs=1))
        work = ctx.enter_context(tc.tile_pool(name="work", bufs=2))
        psA = ctx.enter_context(tc.tile_pool(name="psA", bufs=1, space="PSUM"))
        psB = ctx.enter_context(tc.tile_pool(name="psB", bufs=3, space="PSUM"))
        psT = ctx.enter_context(tc.tile_pool(name="psT", bufs=1, space="PSUM"))
        psC = ctx.enter_context(tc.tile_pool(name="psC", bufs=1, space="PSUM"))
        psD = ctx.enter_context(tc.tile_pool(name="psD", bufs=1, space="PSUM"))

        # ---- critical-path inputs first, on the sync queue ---------------
        qb_s = state.tile([P, NPAIR, 3], F32, tag="qb_s")
        nc.sync.dma_start(out=qb_s[:, :, :],
                          in_=qb[:, :].rearrange("p (i c) -> p i c", c=3))
        ptile = state.tile([P, BPC, 3, NP], F32, tag="ptile")
        for half in range(2):
            nc.sync.dma_start(
                out=ptile[:, 2 * half:2 * half + 2, :, :],
                in_=pts[:, :].rearrange("p (b c j) -> p b c j", b=BPC, c=3)
                [:, 2 * half:2 * half + 2, :, :])

        # ---- constants on the scalar queue -------------------------------
        ident = cpool.tile([P, P], F32, tag="ident")
        nc.gpsimd.dma_start(out=ident[:, :], in_=identf[:, :])
        identb_s = cpool.tile([P, P], BF16, tag="identb_s")
        nc.gpsimd.dma_start(out=identb_s[:, :], in_=identb[:, :])
        onesb_s = cpool.tile([1, P], BF16, tag="onesb_s")
        nc.gpsimd.dma_start(out=onesb_s[:, :], in_=onesb[:, :])
        pbase_s = cpool.tile([P, 1], F32, tag="pbase_s")
        nc.gpsimd.dma_start(out=pbase_s[:, :], in_=pbase[:, :])
        boffs_s = cpool.tile([P, 1], F32, tag="boffs_s")
        nc.gpsimd.dma_start(out=boffs_s[:, :], in_=boffs[:, :])
        mask8_s = cpool.tile([P, 8], F32, tag="mask8_s")
        nc.gpsimd.dma_start(out=mask8_s[:, :], in_=mask8[:, :])
        lrep_s = cpool.tile([P, P], F32, tag="lrep_s")
        nc.gpsimd.dma_start(out=lrep_s[:, :], in_=lrep[:, :])
        erep_s = cpool.tile([NPAIR, NPAIR * P], F32, tag="erep_s")
        nc.gpsimd.dma_start(out=erep_s[:, :], in_=erep[:, :])
        maskr_s = cpool.tile([NPAIR * 8, 8], F32, tag="maskr_s")
        nc.gpsimd.dma_start(out=maskr_s[:, :], in_=maskr[:, :])
        selq_s = cpool.tile([NPAIR * 8, NPAIR], F32, tag="selq_s")
        nc.gpsimd.dma_start(out=selq_s[:, :], in_=selq[:, :])

        # ---- bulk weights on the sync queue (behind pts) -----------------
        w1s, w2s, b1s, b2s = {}, {}, {}, {}
        for t in range(3):
            w1 = wpool.tile([P, 8, H], BF16, tag=f"w1_{t}")
            nc.sync.dma_start(out=w1[:, :, :],
                              in_=wd[f"w1_{t}"][:, :].rearrange(
                                  "p (ch o) -> p ch o", ch=8))
            w2 = wpool.tile([P, 4, H], BF16, tag=f"w2_{t}")
            nc.sync.dma_start(out=w2[:, :, :],
                              in_=wd[f"w2_{t}"][:, :].rearrange(
                                  "p (ch o) -> p ch o", ch=4))
            b1 = wpool.tile([1, H], BF16, tag=f"b1_{t}")
            nc.sync.dma_start(out=b1[:, :], in_=wd[f"b1_{t}"][:, :])
            b2 = wpool.tile([1, H], BF16, tag=f"b2_{t}")
            nc.sync.dma_start(out=b2[:, :], in_=wd[f"b2_{t}"][:, :])
            w1s[t], w2s[t], b1s[t], b2s[t] = w1, w2, b1, b2

        # ---- PE warm-up scratch -----------------------------------------
        dps = psD.tile([NPAIR, H], F32, tag="dummy", name="dps")

        def warm_f32(anchor_ap):
            """Dummy fp32 matmul keyed on anchor_ap to keep PE un-gated."""
            kk = anchor_ap.shape[0]
            nc.tensor.matmul(out=dps[:anchor_ap.shape[1], :P], lhsT=anchor_ap,
                             rhs=ident[:kk, :], start=True, stop=True)

        def warm_bf16(anchor_ap):
            nc.tensor.matmul(out=dps[:anchor_ap.shape[1], :], lhsT=anchor_ap,
                             rhs=w1s[0][:, 0, :], start=True, stop=True)

        # ---- stage A: s = -d2, stage B: per-partition top-8 --------------
        # vector handles pairs [0,1,4,2,6]; gpsimd handles [5,3,7]
        s_all = state.tile([P, NPAIR, NP], F32, tag="s_all")
        v8f = state.tile([P, NPAIR * 8], F32, tag="v8f")

        # per (pair, coord): (p_c - q_c)^2 in one scalar ACT op (bias is
        # the replicated -q as a per-partition AP); vector then just adds
        # the three squares (with a fused negate) and takes the top-8.
        for i in [0, 1, 4, 5, 2, 3, 6, 7]:
            b = i % BPC
            sq = work.tile([P, 3, NP], F32, tag="sq", name=f"sq{i}")
            for c in range(3):
                nc.scalar.activation(out=sq[:, c, :], in_=ptile[:, b, c, :],
                                     func=ACTF.Square,
                                     bias=qb_s[:, i, c:c + 1])
            t12 = work.tile([P, NP], F32, tag="t12", name=f"t12{i}")
            nc.vector.tensor_tensor(out=t12[:, :], in0=sq[:, 0, :],
                                    in1=sq[:, 1, :], op=OP.add)
            nc.vector.scalar_tensor_tensor(
                out=s_all[:, i, :], in0=sq[:, 2, :], scalar=-1.0,
                in1=t12[:, :], op0=OP.mult, op1=OP.subtract)
            nc.vector.max(out=v8f[:, i * 8:(i + 1) * 8], in_=s_all[:, i, :])

        # ---- transpose candidates: [128, 64] -> [64, 128] ----------------
        tvp = psA.tile([NPAIR * 8, P], F32, tag="t64", name="tvp")
        nc.tensor.transpose(out=tvp[:, :], in_=v8f[:, :], identity=ident[:, :])
        tv = state.tile([NPAIR * 8, P], F32, tag="tv")
        nc.vector.tensor_copy(tv[:, :], tvp[:, :])

        # ---- stage C: per-row top-32 of candidates -----------------------
        cv = state.tile([NPAIR * 8, 32], F32, tag="cv")
        for r in range(4):
            sl = cv[:, r * 8:(r + 1) * 8]
            nc.vector.max(out=sl, in_=tv[:, :])
            if r < 3:
                nc.vector.match_replace(out=tv[:, :], in_to_replace=sl,
                                        in_values=tv[:, :], imm_value=NEG)

        # ---- flatten [64,32] -> [8,256] via one masked matmul ------------
        # cvmask[k, r*32+c] = cv[k, c]*(k%8==r); cand = selq.T @ cvmask so
        # cand[q, r*32+c] = cv[q*8+r, c].
        cvmask = state.tile([NPAIR * 8, 8, 32], F32, tag="cvmask")
        nc.vector.tensor_tensor(
            out=cvmask[:, :, :],
            in0=cv[:, :].rearrange("k (a c) -> k a c", a=1).to_broadcast(
                [NPAIR * 8, 8, 32]),
            in1=maskr_s[:, :].rearrange("k (r u) -> k r u", u=1).to_broadcast(
                [NPAIR * 8, 8, 32]),
            op=OP.mult)
        candp = psC.tile([NPAIR, NPAIR * 32], F32, tag="ctr", name="candp")
        nc.tensor.matmul(out=candp[:, :], lhsT=selq_s[:, :],
                         rhs=cvmask[:, :, :].rearrange("k a c -> k (a c)"),
                         start=True, stop=True)
        cand = state.tile([NPAIR, 8 * 32], F32, tag="cand")
        nc.vector.tensor_copy(cand[:, :], candp[:, :])

        # ---- stage D: global top-32 --------------------------------------
        wv = state.tile([NPAIR, 32], F32, tag="wv")
        for r in range(4):
            sl = wv[:, r * 8:(r + 1) * 8]
            nc.vector.max(out=sl, in_=cand[:, :])
            if r < 3:
                nc.vector.match_replace(out=cand[:, :], in_to_replace=sl,
                                        in_values=cand[:, :], imm_value=NEG)

        # ---- broadcast winners via per-pair selector matmuls, pipelined
        # with the finds (pair i's finds start as soon as its own 128x32
        # broadcast lands; emission order matches the find loop) ----------
        wBs = {}
        for q in [0, 1, 4, 5, 2, 3, 6, 7]:
            wbp = psA.tile([P, 32], F32, tag="bc", name=f"wbp{q}")
            nc.tensor.matmul(out=wbp[:, :],
                             lhsT=erep_s[:, q * P:(q + 1) * P],
                             rhs=wv[:, :], start=True, stop=True)
            wB = state.tile([P, 32], F32, tag=f"wB{q}", name=f"wB{q}")
            nc.vector.tensor_copy(wB[:, :], wbp[:, :])
            wBs[q] = wB

        # ---- per 2-batch chunk: index recovery + gather + maxpool --------
        # ju column for (pair, g): h*128 + b2*64 + t*32 + (g//2)*16 + (g%2)*8
        # so that post-transpose partition q = b2*64 + t*32 + w, which is
        # dma_gather's unwrapped slot order (slot i reads table[i%16, i//16])
        # once the masked Lrep matmul rewraps gcl into table[k,j]=gcl[j*16+k]
        # replicated to every 16-partition block.
        jus = [state.tile([P, P], U32, tag=f"ju{h}", name=f"ju{h}")
               for h in range(2)]
        jfs = [state.tile([P, P], F32, tag=f"jf{h}", name=f"jf{h}")
               for h in range(2)]
        gfins = [state.tile([P, 1], F32, tag=f"gfin{h}", name=f"gfin{h}")
                 for h in range(2)]
        gcls = [state.tile([P, 1], F32, tag=f"gcl{h}", name=f"gcl{h}")
                for h in range(2)]
        Xall = state.tile([P, 8, BPC, 2], BF16, tag="Xall")
        gmasks = []
        xgs = []
        for hh in range(2):
            for t in range(2):
                for b2 in range(2):
                    i = t * BPC + 2 * hh + b2
                    for g in range(4):
                        col = b2 * 64 + t * 32 + (g // 2) * 16 + (g % 2) * 8
                        nc.vector.max_index(out=jus[hh][:, col:col + 8],
                                            in_max=wBs[i][:, g * 8:(g + 1) * 8],
                                            in_values=s_all[:, i, :])
            jfh = jfs[hh][:, :]
            nc.vector.tensor_copy(jfh, jus[hh][:, :])
            nc.vector.scalar_tensor_tensor(
                out=jfh, in0=jfh, scalar=1.0,
                in1=pbase_s[:, :].to_broadcast([P, P]),
                op0=OP.mult, op1=OP.add)
            tp = psA.tile([P, P], F32, tag="t64", name=f"tp{hh}")
            nc.tensor.transpose(out=tp[:, :], in_=jfh, identity=ident[:, :])
            nc.vector.tensor_reduce(out=gfins[hh][:, :], in_=tp[:, :],
                                    axis=AX.X, op=OP.min)
            # clamp NOT_FOUND (huge) to N-1 and add per-slot batch offset
            nc.vector.scalar_tensor_tensor(
                out=gcls[hh][:, :], in0=gfins[hh][:, :],
                scalar=float(N - 1), in1=boffs_s[:, :],
                op0=OP.min, op1=OP.add)
            # wrapped+replicated idx table in one masked matmul
            gmask = state.tile([P, 8], F32, tag=f"gmask{hh}", name=f"gmask{hh}")
            gmasks.append(gmask)
            nc.vector.tensor_tensor(
                out=gmask[:, :], in0=gcls[hh][:, :].to_broadcast([P, 8]),
                in1=mask8_s[:, :], op=OP.mult)
            Tp = psA.tile([P, 8], F32, tag="bc", name=f"Tp{hh}")
            nc.tensor.matmul(out=Tp[:, :], lhsT=lrep_s[:, :], rhs=gmask[:, :],
                             start=True, stop=True)
            idx16 = state.tile([P, 8], I16, tag=f"idx16_{hh}", name=f"idx16_{hh}")
            nc.vector.tensor_copy(idx16[:, :], Tp[:, :])
            xg = state.tile([P, 8, P], BF16, tag=f"xg{hh}", name=f"xg{hh}")
            nc.gpsimd.dma_gather(
                xg[:, :, :], feats[hh][:, :], idx16[:, :],
                num_idxs=P, num_idxs_reg=P, elem_size=C, transpose=True)
            xgs.append(xg)

        for hh in range(2):
            nc.vector.tensor_reduce(
                out=Xall[:, :, 2 * hh:2 * hh + 2, :],
                in_=xgs[hh][:, :, :].rearrange("p c8 (b2 t w) -> p c8 b2 t w",
                                               t=2, w=32),
                axis=AX.X, op=OP.max)

        # solid PE warm-up block across the gather wait: first dummy is
        # gated on chunk 1's index table (so the block can't be hoisted
        # early and re-cool), the rest self-chain via WAR on dps and run
        # back-to-back, lifting the HAM clock gate before the MLP.
        warm_f32(gmasks[1][:, :])
        for _ in range(16):
            warm_bf16(identb_s[:, :8])

        # ---- MLPs (bf16) -------------------------------------------------
        def mlp2(t, xin_sl):
            """xin_sl(ch) -> lhsT [128, BPC] bf16; returns psum [BPC, H]."""
            ps1 = psB.tile([BPC, H], F32, tag="mm", name=f"ps1_{t}")
            for ch in range(8):
                nc.tensor.matmul(out=ps1[:, :], lhsT=xin_sl(ch),
                                 rhs=w1s[t][:, ch, :], start=(ch == 0), stop=False)
            nc.tensor.matmul(out=ps1[:, :], lhsT=onesb_s[:1, :BPC],
                             rhs=b1s[t][:1, :], start=False, stop=True)
            h = state.tile([BPC, H], BF16, tag=f"h_{t}")
            nc.scalar.activation(out=h[:, :], in_=ps1[:, :], func=ACTF.Relu)
            hTp = psT.tile([P, 4 * BPC], BF16, tag="tr", name=f"hTp_{t}")
            for ic in range(4):
                nc.tensor.transpose(out=hTp[:, ic * BPC:(ic + 1) * BPC],
                                    in_=h[:, ic * P:(ic + 1) * P],
                                    identity=identb_s[:BPC, :BPC])
            hT = state.tile([P, 4, BPC], BF16, tag=f"hT_{t}")
            nc.vector.tensor_copy(hT[:, :, :],
                                  hTp[:, :].rearrange("p (ic b) -> p ic b", b=BPC))
            ps2 = psB.tile([BPC, H], F32, tag="mm", name=f"ps2_{t}")
            for ic in range(4):
                nc.tensor.matmul(out=ps2[:, :], lhsT=hT[:, ic, :],
                                 rhs=w2s[t][:, ic, :], start=(ic == 0), stop=False)
            nc.tensor.matmul(out=ps2[:, :], lhsT=onesb_s[:1, :BPC],
                             rhs=b2s[t][:1, :], start=False, stop=True)
            return ps2

        cT = state.tile([P, 8, BPC], BF16, tag="cT")
        cTp = psC.tile([P, 8 * BPC], BF16, tag="ctr", name="cTp")
        for t in range(2):
            ps2 = mlp2(t, lambda ch: Xall[:, ch, :, t])
            o = state.tile([BPC, H], BF16, tag=f"o_{t}")
            nc.scalar.activation(out=o[:, :], in_=ps2[:, :], func=ACTF.Copy)
            for ic in range(4):
                nc.tensor.transpose(
                    out=cTp[:, (t * 4 + ic) * BPC:(t * 4 + ic + 1) * BPC],
                    in_=o[:, ic * P:(ic + 1) * P],
                    identity=identb_s[:BPC, :BPC])
        nc.vector.tensor_copy(cT[:, :, :],
                              cTp[:, :].rearrange("p (ic b) -> p ic b", b=BPC))

        ps_f = mlp2(2, lambda ch: cT[:, ch, :])
        res = state.tile([BPC, OUT], F32, tag="res")
        nc.vector.tensor_copy(res[:, :], ps_f[:, :])
        nc.sync.dma_start(out=out[:, :], in_=res[:, :])


_NC_CACHE = None


def _get_nc():
    global _NC_CACHE
    if _NC_CACHE is None:
        _NC_CACHE = build_nc()
    return _NC_CACHE


def _consts():
    identf = np.eye(P, dtype=np.float32)
    identb = np.eye(P).astype(BF)
    onesb = np.ones((1, P)).astype(BF)
    pbase = (np.arange(P, dtype=np.float32) * NP).reshape(P, 1)
    boffs = ((np.arange(P) // 64) * N).astype(np.float32).reshape(P, 1)
    mask8 = (np.arange(P)[:, None] // 16 == np.arange(8)[None, :]).astype(
        np.float32)
    lrep = (np.arange(P)[:, None] % 16 == np.arange(P)[None, :] % 16).astype(
        np.float32)
    erep = np.zeros((NPAIR, NPAIR * P), dtype=np.float32)
    for q in range(NPAIR):
        erep[q, q * P:(q + 1) * P] = 1.0
    maskr = (np.arange(NPAIR * 8)[:, None] % 8 == np.arange(8)[None, :]
             ).astype(np.float32)
    selq = (np.arange(NPAIR * 8)[:, None] // 8 == np.arange(NPAIR)[None, :]
            ).astype(np.float32)
    return {"identf": identf, "identb": identb, "onesb": onesb,
            "pbase": pbase, "boffs": boffs, "mask8": mask8,
            "lrep": lrep, "erep": erep, "maskr": maskr, "selq": selq}


def build_in_maps(points_xyz, point_features, joint_origin, drag_point,
                  jw1, jb1, jw2, jb2, dw1, db1, dw2, db2, fw1, fb1, fw2, fb2):
    from concurrent.futures import ThreadPoolExecutor

    wmap = {}
    for t, (w1, b1, w2, b2) in enumerate([(jw1, jb1, jw2, jb2),
                                          (dw1, db1, dw2, db2),
                                          (fw1, fb1, fw2, fb2)]):
        w1 = np.asarray(w1, dtype=np.float32)
        w2 = np.asarray(w2, dtype=np.float32)
        nch = w1.shape[0] // P
        wmap[f"w1_{t}"] = np.ascontiguousarray(
            w1.reshape(nch, P, H).transpose(1, 0, 2).reshape(P, nch * H)
        ).astype(BF)
        wmap[f"w2_{t}"] = np.ascontiguousarray(
            w2.reshape(4, P, H).transpose(1, 0, 2).reshape(P, 4 * H)
        ).astype(BF)
        wmap[f"b1_{t}"] = np.asarray(b1, dtype=np.float32).reshape(1, H).astype(BF)
        wmap[f"b2_{t}"] = np.asarray(b2, dtype=np.float32).reshape(1, H).astype(BF)
    wmap.update(_consts())

    pxyz = np.asarray(points_xyz, dtype=np.float32)
    pf = np.asarray(point_features)
    qj = np.asarray(joint_origin, dtype=np.float32)
    qd = np.asarray(drag_point, dtype=np.float32)

    def feats_half(args):
        c, hhalf = args
        buf = np.empty((2 * N, C), dtype=BF)
        for b2 in range(2):
            gb = c * BPC + 2 * hhalf + b2
            buf[b2 * N:(b2 + 1) * N] = pf[gb].T.astype(BF)
        return buf

    with ThreadPoolExecutor(max_workers=16) as ex:
        fhalves = list(ex.map(feats_half,
                              [(c, hh) for c in range(NCORES) for hh in range(2)]))

    in_maps = []
    for c in range(NCORES):
        sl = slice(c * BPC, (c + 1) * BPC)
        ptsc = np.ascontiguousarray(
            pxyz[sl].reshape(BPC, P, NP, 3).transpose(1, 0, 3, 2)
        ).reshape(P, BPC * NP * 3)
        qcat = -np.concatenate([qj[sl], qd[sl]], axis=0).reshape(-1)
        qbc = np.ascontiguousarray(
            np.broadcast_to(qcat[None, :], (P, NPAIR * 3)))
        m = {"pts": ptsc, "qb": qbc,
             "feats0": fhalves[c * 2], "feats1": fhalves[c * 2 + 1]}
        m.update(wmap)
        in_maps.append(m)
    return in_maps


def kernel(**inputs):
    from concourse import bass_utils

    nc = _get_nc()
    in_maps = build_in_maps(**inputs)
    res = bass_utils.run_bass_kernel_spmd(nc, in_maps, core_ids=list(range(NCORES)))
    return np.concatenate([r["out"] for r in res.results], axis=0)


# revision 15
# speedup vs baseline: 1.0139x; 1.0009x over previous
"""Trainium2 Bass kernel for LocalFeatureSamplerV10 (retrieval_knn).

Full-input contract: kernel(**inputs) takes the complete unsharded numpy
inputs and returns the full [32, 512] output. Internally shards the batch
dim over 8 NeuronCores (4 batches/core), replicating the MLP weights.

Optimizations vs the original fp32 baseline (163us -> ~76us measured):
  * All MLP matmuls/transposes in bf16 (weights shipped bf16 from host);
    fp32 512-col matmuls in LOW_HIGH mode were ~1060ns each vs ~380 warm.
  * Host ships precomputed constants (replicated queries, identities,
    pbase/boffs columns, selector/replication matrices) removing the
    on-device iota/broadcast dependency chain at the head of the kernel.
  * DMA queue discipline: bulk weight traffic on the sync (qSPDynamicHW)
    queue only, constants on the gpsimd queue, nothing critical behind
    megabyte transfers; points staged in 2 DMAs so batch 0/1 distance
    compute starts while batches 2/3 are still in flight.
  * Stage C->D flatten and the winner broadcast are masked/selector PE
    matmuls instead of SBUF-SBUF DMA hops; the broadcast is pipelined
    per pair so FIND_INDEX8 starts as soon as pair 0's row lands.
  * Index tables built with one masked matmul (Lrep.T @ (gcl*mask8)) that
    also replicates them into every 16-partition block for dma_gather.
  * Features shipped bf16 as two [2N, C] stacks; the K-row gather is two
    dma_gather(transpose=True) ops that land channels-on-partitions
    directly (replacing 8 serial indirect DMAs + 32 fp32 PE transposes),
    with FIND outputs written to permuted columns so the cross-partition
    min produces indices already in dma_gather's wrapped table order.
  * A gated self-chained dummy-matmul block keeps the PE HAM clock gate
    open across the gather wait so the MLP runs at full clock.

Per-core algorithm (4 batches x 2 queries = 8 "pairs", pair = t*4 + b):
  1. s = -||p - q||^2 laid out [128 part, 128] per pair (point n = p*128+j).
  2. Top-32 per pair: per-partition top-8 (max8) -> PE-transpose candidates
     -> per-row top-32 (max + match_replace rounds) -> masked-matmul
     flatten -> global top-32 with pairs stacked on partitions (bit-exact).
  3. Indices via max_index against the original s rows + p*128, cross-
     partition min via PE transpose + reduce_min, clamped, + batch offset.
  4. Two dma_gather(transpose=True) of 128 rows each from the bf16 feature
     stacks; vector reduce_max over K -> X [128ch, 8chhi, b, t] bf16.
  5. MLPs as bf16 PE matmuls with batch on partitions; biases folded in as
     rank-1 ones-matmuls; PE transposes between layers; fp32 output.
"""

import numpy as np
import ml_dtypes

import concourse.bass as bass
from concourse import bacc
import concourse.mybir as mybir
import concourse.tile as tile

B, N, C, K, OUT = 32, 16384, 1024, 32, 512
H = 512
NCORES = 8
BPC = B // NCORES          # batches per core
P = 128
NP = N // P                # 128 points per partition
NPAIR = 2 * BPC            # 8 (pair = t*BPC + b; 0-3 joint, 4-7 drag)
F32 = mybir.dt.float32
BF16 = mybir.dt.bfloat16
U32 = mybir.dt.uint32
I16 = mybir.dt.int16
NEG = -3.0e38

AX = mybir.AxisListType
OP = mybir.AluOpType
ACTF = mybir.ActivationFunctionType

BF = ml_dtypes.bfloat16


def build_nc():
    nc = bacc.Bacc(trn_type="TRN2")

    pts = nc.dram_tensor("pts", [P, BPC * NP * 3], F32, kind="ExternalInput")
    qb = nc.dram_tensor("qb", [P, NPAIR * 3], F32, kind="ExternalInput")
    identf = nc.dram_tensor("identf", [P, P], F32, kind="ExternalInput")
    identb = nc.dram_tensor("identb", [P, P], BF16, kind="ExternalInput")
    onesb = nc.dram_tensor("onesb", [1, P], BF16, kind="ExternalInput")
    pbase = nc.dram_tensor("pbase", [P, 1], F32, kind="ExternalInput")
    boffs = nc.dram_tensor("boffs", [P, 1], F32, kind="ExternalInput")
    mask8 = nc.dram_tensor("mask8", [P, 8], F32, kind="ExternalInput")
    lrep = nc.dram_tensor("lrep", [P, P], F32, kind="ExternalInput")
    erep = nc.dram_tensor("erep", [NPAIR, NPAIR * P], F32, kind="ExternalInput")
    maskr = nc.dram_tensor("maskr", [NPAIR * 8, 8], F32, kind="ExternalInput")
    selq = nc.dram_tensor("selq", [NPAIR * 8, NPAIR], F32, kind="ExternalInput")
    feats = [nc.dram_tensor(f"feats{h}", [2 * N, C], BF16, kind="ExternalInput")
             for h in range(2)]
    wd = {}
    for t in range(3):
        wd[f"w1_{t}"] = nc.dram_tensor(f"w1_{t}", [P, 8 * H], BF16,
                                       kind="ExternalInput")
        wd[f"w2_{t}"] = nc.dram_tensor(f"w2_{t}", [P, 4 * H], BF16,
                                       kind="ExternalInput")
        wd[f"b1_{t}"] = nc.dram_tensor(f"b1_{t}", [1, H], BF16,
                                       kind="ExternalInput")
        wd[f"b2_{t}"] = nc.dram_tensor(f"b2_{t}", [1, H], BF16,
                                       kind="ExternalInput")
    out = nc.dram_tensor("out", [BPC, OUT], F32, kind="ExternalOutput")

    with tile.TileContext(nc) as tc:
        _body(tc, nc, pts, qb, identf, identb, onesb, pbase, boffs,
              mask8, lrep, erep, maskr, selq, feats, wd, out)
    nc.compile()
    return nc


def _body(tc, nc, pts, qb, identf, identb, onesb, pbase, boffs,
          mask8, lrep, erep, maskr, selq, feats, wd, out):
    from contextlib import ExitStack
    with ExitStack() as ctx:
        cpool = ctx.enter_context(tc.tile_pool(name="const", bufs=1))
        wpool = ctx.enter_context(tc.tile_pool(name="weights", bufs=1))
        state = ctx.enter_context(tc.tile_pool(name="state", bufs=1))
        work = ctx.enter_context(tc.tile_pool(name="work", bufs=2))
        psA = ctx.enter_context(tc.tile_pool(name="psA", bufs=1, space="PSUM"))
        psB = ctx.enter_context(tc.tile_pool(name="psB", bufs=3, space="PSUM"))
        psT = ctx.enter_context(tc.tile_pool(name="psT", bufs=1, space="PSUM"))
        psC = ctx.enter_context(tc.tile_pool(name="psC", bufs=1, space="PSUM"))
        psD = ctx.enter_context(tc.tile_pool(name="psD", bufs=1, space="PSUM"))

        # ---- critical-path inputs first, on the sync queue ---------------
        qb_s = state.tile([P, NPAIR, 3], F32, tag="qb_s")
        nc.sync.dma_start(out=qb_s[:, :, :],
                          in_=qb[:, :].rearrange("p (i c) -> p i c", c=3))
        ptile = state.tile([P, BPC, NP * 3], F32, tag="ptile")
        for half in range(2):
            nc.sync.dma_start(
                out=ptile[:, 2 * half:2 * half + 2, :],
                in_=pts[:, :].rearrange("p (b x) -> p b x", b=BPC)
                [:, 2 * half:2 * half + 2, :])

        # ---- constants on the scalar queue -------------------------------
        ident = cpool.tile([P, P], F32, tag="ident")
        nc.gpsimd.dma_start(out=ident[:, :], in_=identf[:, :])
        identb_s = cpool.tile([P, P], BF16, tag="identb_s")
        nc.gpsimd.dma_start(out=identb_s[:, :], in_=identb[:, :])
        onesb_s = cpool.tile([1, P], BF16, tag="onesb_s")
        nc.gpsimd.dma_start(out=onesb_s[:, :], in_=onesb[:, :])
        pbase_s = cpool.tile([P, 1], F32, tag="pbase_s")
        nc.gpsimd.dma_start(out=pbase_s[:, :], in_=pbase[:, :])
        boffs_s = cpool.tile([P, 1], F32, tag="boffs_s")
        nc.gpsimd.dma_start(out=boffs_s[:, :], in_=boffs[:, :])
        mask8_s = cpool.tile([P, 8], F32, tag="mask8_s")
        nc.gpsimd.dma_start(out=mask8_s[:, :], in_=mask8[:, :])
        lrep_s = cpool.tile([P, P], F32, tag="lrep_s")
        nc.gpsimd.dma_start(out=lrep_s[:, :], in_=lrep[:, :])
        erep_s = cpool.tile([NPAIR, NPAIR * P], F32, tag="erep_s")
        nc.gpsimd.dma_start(out=erep_s[:, :], in_=erep[:, :])
        maskr_s = cpool.tile([NPAIR * 8, 8], F32, tag="maskr_s")
        nc.gpsimd.dma_start(out=maskr_s[:, :], in_=maskr[:, :])
        selq_s = cpool.tile([NPAIR * 8, NPAIR], F32, tag="selq_s")
        nc.gpsimd.dma_start(out=selq_s[:, :], in_=selq[:, :])

        # ---- bulk weights on the sync queue (behind pts) -----------------
        w1s, w2s, b1s, b2s = {}, {}, {}, {}
        for t in range(3):
            w1 = wpool.tile([P, 8, H], BF16, tag=f"w1_{t}")
            nc.sync.dma_start(out=w1[:, :, :],
                              in_=wd[f"w1_{t}"][:, :].rearrange(
                                  "p (ch o) -> p ch o", ch=8))
            w2 = wpool.tile([P, 4, H], BF16, tag=f"w2_{t}")
            nc.sync.dma_start(out=w2[:, :, :],
                              in_=wd[f"w2_{t}"][:, :].rearrange(
                                  "p (ch o) -> p ch o", ch=4))
            b1 = wpool.tile([1, H], BF16, tag=f"b1_{t}")
            nc.sync.dma_start(out=b1[:, :], in_=wd[f"b1_{t}"][:, :])
            b2 = wpool.tile([1, H], BF16, tag=f"b2_{t}")
            nc.sync.dma_start(out=b2[:, :], in_=wd[f"b2_{t}"][:, :])
            w1s[t], w2s[t], b1s[t], b2s[t] = w1, w2, b1, b2

        # ---- PE warm-up scratch -----------------------------------------
        dps = psD.tile([NPAIR, H], F32, tag="dummy", name="dps")

        def warm_f32(anchor_ap):
            """Dummy fp32 matmul keyed on anchor_ap to keep PE un-gated."""
            kk = anchor_ap.shape[0]
            nc.tensor.matmul(out=dps[:anchor_ap.shape[1], :P], lhsT=anchor_ap,
                             rhs=ident[:kk, :], start=True, stop=True)

        def warm_bf16(anchor_ap):
            nc.tensor.matmul(out=dps[:anchor_ap.shape[1], :], lhsT=anchor_ap,
                             rhs=w1s[0][:, 0, :], start=True, stop=True)

        # ---- stage A: s = -d2, stage B: per-partition top-8 --------------
        # vector handles pairs [0,1,4,2,6]; gpsimd handles [5,3,7]
        s_all = state.tile([P, NPAIR, NP], F32, tag="s_all")
        v8f = state.tile([P, NPAIR * 8], F32, tag="v8f")

        def dist_sub(eng, i):
            b = i % BPC
            pv = ptile[:, b, :].rearrange("p (j c) -> p j c", c=3)
            diff = work.tile([P, NP * 3], F32, tag="diff", name=f"diff{i}")
            eng.tensor_sub(
                out=diff[:, :].rearrange("p (j c) -> p j c", c=3), in0=pv,
                in1=qb_s[:, i:i + 1, :].to_broadcast([P, NP, 3]))
            return diff

        def dist_sqred(eng, i, diff):
            sq = work.tile([P, NP * 3], F32, tag="sq", name=f"sq{i}")
            nc.scalar.square(out=sq[:, :], in_=diff[:, :])
            eng.tensor_reduce(out=s_all[:, i, :],
                              in_=sq[:, :].rearrange("p (j c) -> p j c", c=3),
                              axis=AX.X, op=OP.add, negate=True)

        for i in [0, 1, 4, 5, 2, 3, 6, 7]:
            d = dist_sub(nc.vector, i)
            dist_sqred(nc.vector, i, d)
            nc.vector.max(out=v8f[:, i * 8:(i + 1) * 8], in_=s_all[:, i, :])

        # ---- transpose candidates: [128, 64] -> [64, 128] ----------------
        tvp = psA.tile([NPAIR * 8, P], F32, tag="t64", name="tvp")
        nc.tensor.transpose(out=tvp[:, :], in_=v8f[:, :], identity=ident[:, :])
        tv = state.tile([NPAIR * 8, P], F32, tag="tv")
        nc.vector.tensor_copy(tv[:, :], tvp[:, :])

        # ---- stage C: per-row top-32 of candidates -----------------------
        cv = state.tile([NPAIR * 8, 32], F32, tag="cv")
        for r in range(4):
            sl = cv[:, r * 8:(r + 1) * 8]
            nc.vector.max(out=sl, in_=tv[:, :])
            if r < 3:
                nc.vector.match_replace(out=tv[:, :], in_to_replace=sl,
                                        in_values=tv[:, :], imm_value=NEG)

        # ---- flatten [64,32] -> [8,256] via one masked matmul ------------
        # cvmask[k, r*32+c] = cv[k, c]*(k%8==r); cand = selq.T @ cvmask so
        # cand[q, r*32+c] = cv[q*8+r, c].
        cvmask = state.tile([NPAIR * 8, 8, 32], F32, tag="cvmask")
        nc.vector.tensor_tensor(
            out=cvmask[:, :, :],
            in0=cv[:, :].rearrange("k (a c) -> k a c", a=1).to_broadcast(
                [NPAIR * 8, 8, 32]),
            in1=maskr_s[:, :].rearrange("k (r u) -> k r u", u=1).to_broadcast(
                [NPAIR * 8, 8, 32]),
            op=OP.mult)
        candp = psC.tile([NPAIR, NPAIR * 32], F32, tag="ctr", name="candp")
        nc.tensor.matmul(out=candp[:, :], lhsT=selq_s[:, :],
                         rhs=cvmask[:, :, :].rearrange("k a c -> k (a c)"),
                         start=True, stop=True)
        cand = state.tile([NPAIR, 8 * 32], F32, tag="cand")
        nc.vector.tensor_copy(cand[:, :], candp[:, :])

        # ---- stage D: global top-32 --------------------------------------
        wv = state.tile([NPAIR, 32], F32, tag="wv")
        for r in range(4):
            sl = wv[:, r * 8:(r + 1) * 8]
            nc.vector.max(out=sl, in_=cand[:, :])
            if r < 3:
                nc.vector.match_replace(out=cand[:, :], in_to_replace=sl,
                                        in_values=cand[:, :], imm_value=NEG)

        # ---- broadcast winners via per-pair selector matmuls, pipelined
        # with the finds (pair i's finds start as soon as its own 128x32
        # broadcast lands; emission order matches the find loop) ----------
        wBs = {}
        for q in [0, 1, 4, 5, 2, 3, 6, 7]:
            wbp = psA.tile([P, 32], F32, tag="bc", name=f"wbp{q}")
            nc.tensor.matmul(out=wbp[:, :],
                             lhsT=erep_s[:, q * P:(q + 1) * P],
                             rhs=wv[:, :], start=True, stop=True)
            wB = state.tile([P, 32], F32, tag=f"wB{q}", name=f"wB{q}")
            nc.vector.tensor_copy(wB[:, :], wbp[:, :])
            wBs[q] = wB

        # ---- per 2-batch chunk: index recovery + gather + maxpool --------
        # ju column for (pair, g): h*128 + b2*64 + t*32 + (g//2)*16 + (g%2)*8
        # so that post-transpose partition q = b2*64 + t*32 + w, which is
        # dma_gather's unwrapped slot order (slot i reads table[i%16, i//16])
        # once the masked Lrep matmul rewraps gcl into table[k,j]=gcl[j*16+k]
        # replicated to every 16-partition block.
        ju = state.tile([P, 2 * P], U32, tag="ju")
        jf = state.tile([P, 2 * P], F32, tag="jf")
        gfin = state.tile([P, 2], F32, tag="gfin")
        gcl = state.tile([P, 2], F32, tag="gcl")
        Xall = state.tile([P, 8, BPC, 2], BF16, tag="Xall")
        gmasks = []
        xgs = []
        for hh in range(2):
            for t in range(2):
                for b2 in range(2):
                    i = t * BPC + 2 * hh + b2
                    for g in range(4):
                        col = hh * 128 + b2 * 64 + t * 32 + (g // 2) * 16 + (g % 2) * 8
                        nc.vector.max_index(out=ju[:, col:col + 8],
                                            in_max=wBs[i][:, g * 8:(g + 1) * 8],
                                            in_values=s_all[:, i, :])
            jfh = jf[:, hh * P:(hh + 1) * P]
            nc.vector.tensor_copy(jfh, ju[:, hh * P:(hh + 1) * P])
            nc.vector.scalar_tensor_tensor(
                out=jfh, in0=jfh, scalar=1.0,
                in1=pbase_s[:, :].to_broadcast([P, P]),
                op0=OP.mult, op1=OP.add)
            tp = psA.tile([P, P], F32, tag="t64", name=f"tp{hh}")
            nc.tensor.transpose(out=tp[:, :], in_=jfh, identity=ident[:, :])
            nc.vector.tensor_reduce(out=gfin[:, hh:hh + 1], in_=tp[:, :],
                                    axis=AX.X, op=OP.min)
            # clamp NOT_FOUND (huge) to N-1 and add per-slot batch offset
            nc.vector.scalar_tensor_tensor(
                out=gcl[:, hh:hh + 1], in0=gfin[:, hh:hh + 1],
                scalar=float(N - 1), in1=boffs_s[:, :],
                op0=OP.min, op1=OP.add)
            # wrapped+replicated idx table in one masked matmul
            gmask = state.tile([P, 8], F32, tag=f"gmask{hh}", name=f"gmask{hh}")
            gmasks.append(gmask)
            nc.vector.tensor_tensor(
                out=gmask[:, :], in0=gcl[:, hh:hh + 1].to_broadcast([P, 8]),
                in1=mask8_s[:, :], op=OP.mult)
            Tp = psA.tile([P, 8], F32, tag="bc", name=f"Tp{hh}")
            nc.tensor.matmul(out=Tp[:, :], lhsT=lrep_s[:, :], rhs=gmask[:, :],
                             start=True, stop=True)
            idx16 = state.tile([P, 8], I16, tag=f"idx16_{hh}", name=f"idx16_{hh}")
            nc.vector.tensor_copy(idx16[:, :], Tp[:, :])
            xg = state.tile([P, 8, P], BF16, tag=f"xg{hh}", name=f"xg{hh}")
            nc.gpsimd.dma_gather(
                xg[:, :, :], feats[hh][:, :], idx16[:, :],
                num_idxs=P, num_idxs_reg=P, elem_size=C, transpose=True)
            xgs.append(xg)

        for hh in range(2):
            nc.vector.tensor_reduce(
                out=Xall[:, :, 2 * hh:2 * hh + 2, :],
                in_=xgs[hh][:, :, :].rearrange("p c8 (b2 t w) -> p c8 b2 t w",
                                               t=2, w=32),
                axis=AX.X, op=OP.max)

        # solid PE warm-up block across the gather wait: first dummy is
        # gated on chunk 1's index table (so the block can't be hoisted
        # early and re-cool), the rest self-chain via WAR on dps and run
        # back-to-back, lifting the HAM clock gate before the MLP.
        warm_f32(gmasks[1][:, :])
        for _ in range(16):
            warm_bf16(identb_s[:, :8])

        # ---- MLPs (bf16) -------------------------------------------------
        def mlp2(t, xin_sl):
            """xin_sl(ch) -> lhsT [128, BPC] bf16; returns psum [BPC, H]."""
            ps1 = psB.tile([BPC, H], F32, tag="mm", name=f"ps1_{t}")
            for ch in range(8):
                nc.tensor.matmul(out=ps1[:, :], lhsT=xin_sl(ch),
                                 rhs=w1s[t][:, ch, :], start=(ch == 0), stop=False)
            nc.tensor.matmul(out=ps1[:, :], lhsT=onesb_s[:1, :BPC],
                             rhs=b1s[t][:1, :], start=False, stop=True)
            h = state.tile([BPC, H], BF16, tag=f"h_{t}")
            nc.scalar.activation(out=h[:, :], in_=ps1[:, :], func=ACTF.Relu)
            hTp = psT.tile([P, 4 * BPC], BF16, tag="tr", name=f"hTp_{t}")
            for ic in range(4):
                nc.tensor.transpose(out=hTp[:, ic * BPC:(ic + 1) * BPC],
                                    in_=h[:, ic * P:(ic + 1) * P],
                                    identity=identb_s[:BPC, :BPC])
            hT = state.tile([P, 4, BPC], BF16, tag=f"hT_{t}")
            nc.vector.tensor_copy(hT[:, :, :],
                                  hTp[:, :].rearrange("p (ic b) -> p ic b", b=BPC))
            ps2 = psB.tile([BPC, H], F32, tag="mm", name=f"ps2_{t}")
            for ic in range(4):
                nc.tensor.matmul(out=ps2[:, :], lhsT=hT[:, ic, :],
                                 rhs=w2s[t][:, ic, :], start=(ic == 0), stop=False)
            nc.tensor.matmul(out=ps2[:, :], lhsT=onesb_s[:1, :BPC],
                             rhs=b2s[t][:1, :], start=False, stop=True)
            return ps2

        cT = state.tile([P, 8, BPC], BF16, tag="cT")
        cTp = psC.tile([P, 8 * BPC], BF16, tag="ctr", name="cTp")
        for t in range(2):
            ps2 = mlp2(t, lambda ch: Xall[:, ch, :, t])
            o = state.tile([BPC, H], BF16, tag=f"o_{t}")
            nc.scalar.activation(out=o[:, :], in_=ps2[:, :], func=ACTF.Copy)
            for ic in range(4):
                nc.tensor.transpose(
                    out=cTp[:, (t * 4 + ic) * BPC:(t * 4 + ic + 1) * BPC],
                    in_=o[:, ic * P:(ic + 1) * P],
                    identity=identb_s[:BPC, :BPC])
        nc.vector.tensor_copy(cT[:, :, :],
                              cTp[:, :].rearrange("p (ic b) -> p ic b", b=BPC))

        ps_f = mlp2(2, lambda ch: cT[:, ch, :])
        res = state.tile([BPC, OUT], F32, tag="res")
        nc.vector.tensor_copy(res[:, :], ps_f[:, :])
        nc.sync.dma_start(out=out[:, :], in_=res[:, :])


_NC_CACHE = None


def _get_nc():
    global _NC_CACHE
    if _NC_CACHE is None:
        _NC_CACHE = build_nc()
    return _NC_CACHE


def _consts():
    identf = np.eye(P, dtype=np.float32)
    identb = np.eye(P).astype(BF)
    onesb = np.ones((1, P)).astype(BF)
    pbase = (np.arange(P, dtype=np.float32) * NP).reshape(P, 1)
    boffs = ((np.arange(P) // 64) * N).astype(np.float32).reshape(P, 1)
    mask8 = (np.arange(P)[:, None] // 16 == np.arange(8)[None, :]).astype(
        np.float32)
    lrep = (np.arange(P)[:, None] % 16 == np.arange(P)[None, :] % 16).astype(
        np.float32)
    erep = np.zeros((NPAIR, NPAIR * P), dtype=np.float32)
    for q in range(NPAIR):
        erep[q, q * P:(q + 1) * P] = 1.0
    maskr = (np.arange(NPAIR * 8)[:, None] % 8 == np.arange(8)[None, :]
             ).astype(np.float32)
    selq = (np.arange(NPAIR * 8)[:, None] // 8 == np.arange(NPAIR)[None, :]
            ).astype(np.float32)
    return {"identf": identf, "identb": identb, "onesb": onesb,
            "pbase": pbase, "boffs": boffs, "mask8": mask8,
            "lrep": lrep, "erep": erep, "maskr": maskr, "selq": selq}


def build_in_maps(points_xyz, point_features, joint_origin, drag_point,
                  jw1, jb1, jw2, jb2, dw1, db1, dw2, db2, fw1, fb1, fw2, fb2):
    from concurrent.futures import ThreadPoolExecutor

    wmap = {}
    for t, (w1, b1, w2, b2) in enumerate([(jw1, jb1, jw2, jb2),
                                          (dw1, db1, dw2, db2),
                                          (fw1, fb1, fw2, fb2)]):
        w1 = np.asarray(w1, dtype=np.float32)
        w2 = np.asarray(w2, dtype=np.float32)
        nch = w1.shape[0] // P
        wmap[f"w1_{t}"] = np.ascontiguousarray(
            w1.reshape(nch, P, H).transpose(1, 0, 2).reshape(P, nch * H)
        ).astype(BF)
        wmap[f"w2_{t}"] = np.ascontiguousarray(
            w2.reshape(4, P, H).transpose(1, 0, 2).reshape(P, 4 * H)
        ).astype(BF)
        wmap[f"b1_{t}"] = np.asarray(b1, dtype=np.float32).reshape(1, H).astype(BF)
        wmap[f"b2_{t}"] = np.asarray(b2, dtype=np.float32).reshape(1, H).astype(BF)
    wmap.update(_consts())

    pxyz = np.asarray(points_xyz, dtype=np.float32)
    pf = np.asarray(point_features)
    qj = np.asarray(joint_origin, dtype=np.float32)
    qd = np.asarray(drag_point, dtype=np.float32)

    def feats_half(args):
        c, hhalf = args
        buf = np.empty((2 * N, C), dtype=BF)
        for b2 in range(2):
            gb = c * BPC + 2 * hhalf + b2
            buf[b2 * N:(b2 + 1) * N] = pf[gb].T.astype(BF)
        return buf

    with ThreadPoolExecutor(max_workers=16) as ex:
        fhalves = list(ex.map(feats_half,
                              [(c, hh) for c in range(NCORES) for hh in range(2)]))

    in_maps = []
    for c in range(NCORES):
        sl = slice(c * BPC, (c + 1) * BPC)
        ptsc = np.ascontiguousarray(
            pxyz[sl].reshape(BPC, P, NP, 3).transpose(1, 0, 2, 3)
        ).reshape(P, BPC * NP * 3)
        qcat = np.concatenate([qj[sl], qd[sl]], axis=0).reshape(-1)
        qbc = np.ascontiguousarray(
            np.broadcast_to(qcat[None, :], (P, NPAIR * 3)))
        m = {"pts": ptsc, "qb": qbc,
             "feats0": fhalves[c * 2], "feats1": fhalves[c * 2 + 1]}
        m.update(wmap)
        in_maps.append(m)
    return in_maps


def kernel(**inputs):
    from concourse import bass_utils

    nc = _get_nc()
    in_maps = build_in_maps(**inputs)
    res = bass_utils.run_bass_kernel_spmd(nc, in_maps, core_ids=list(range(NCORES)))
    return np.concatenate([r["out"] for r in res.results], axis=0)
